# revision 10
# baseline (speedup 1.0000x reference)
"""DeepseekMoE (T=1024,H=1024,E=64,K=6,G=8,TG=4,F=512,NSH=2) on 8 trn2 cores.

Strategy: expert-parallel (8 experts/core), full x replicated to every core.
 - routing (gate matmul + grouped top-k) computed data-parallel (128 tok/core)
 - AllGather of (ids, weights) -> every core builds dispatch info for its
   8 experts: per-(token,k) destination slot = expert_rank (via PE lower-tri
   cumsum matmul), scattered as tiny [w, tokid] rows into a per-core table
 - per expert: indirect-gather x rows, PE transpose, fp32 matmuls
   (x@w13 -> silu*mul -> @w2), weight-scale on ACT, indirect scatter-ADD
   (CCE) into a per-core partial y [1024,1024]
 - shared expert computed on the core's own 128-token slice
 - ReduceScatter(add) over 8 cores -> each core's [128,1024] slice,
   + shared -> output
"""

import sys

sys.path.insert(0, "/opt/trn_rl_repo")

import numpy as np

import concourse.bass as bass
import concourse.bacc as bacc
import concourse.tile as tile
import concourse.mybir as mybir
from concourse import bass_utils

F32 = mybir.dt.float32
I32 = mybir.dt.int32
U32 = mybir.dt.uint32
AF = mybir.ActivationFunctionType
OP = mybir.AluOpType

N_CORES = 8
T, H, E, K, G, TG, F, NSH = 1024, 1024, 64, 6, 8, 4, 512, 2
SCALE = 2.5
ELOC = E // N_CORES          # experts per core
TSH = T // N_CORES           # tokens per core
CAP = 256                    # per-expert capacity (multiple of 128)
CTILES = CAP // 128
DUMP = ELOC * CAP            # dump row for non-local / overflow slots
NEG = -1.0e30

# consts layout (columns of the [128, 392] per-core constant input)
C_LT = 0        # 128 cols: LT[a,b] = 1.0 if a <= b (lhsT of lower-tri cumsum)
C_ONES = 128    # 128 cols: all ones (lhsT of column-broadcast colsum)
C_IDEN = 256    # 64+64: identity is NOT needed as 128 cols; see below
# we do need a full 128x128 identity for PE transpose:
# layout: [0:128]=LT, [128:256]=ONES, [256:384]=IDENTITY, then:
C_IOTA64 = 384  # 64 cols: iota64 replicated on all partitions
C_BIAS = 448    # 64 cols: bias replicated on all partitions
C_IOTAP = 512   # 1 col: partition index 0..127
C_NEGOFF = 513  # 1 col: -8*core
C_NEGBIG = 514  # 1 col: -1e30
C_DUMP = 515    # 1 col: DUMP
C_COLS = 516

_CACHE = {}


def build_nc(phase: int = 6):
    nc = bacc.Bacc("TRN2", target_bir_lowering=False, debug=False,
                   num_devices=N_CORES)
    x = nc.dram_tensor("x", [T, H], F32, kind="ExternalInput")
    xsh = nc.dram_tensor("xsh", [TSH, H], F32, kind="ExternalInput")
    gwt = nc.dram_tensor("gwt", [H, E], F32, kind="ExternalInput")
    w13 = nc.dram_tensor("w13", [ELOC, H, 2 * F], F32, kind="ExternalInput")
    w2 = nc.dram_tensor("w2", [ELOC, F, H], F32, kind="ExternalInput")
    sw13 = nc.dram_tensor("sw13", [H, 2 * F * NSH], F32, kind="ExternalInput")
    sw2 = nc.dram_tensor("sw2", [F * NSH, H], F32, kind="ExternalInput")
    consts = nc.dram_tensor("consts", [128, C_COLS], F32,
                            kind="ExternalInput")
    yout = nc.dram_tensor("y", [TSH, H], F32, kind="ExternalOutput")

    FSH = F * NSH  # 1024

    with tile.TileContext(nc) as tc:
        with tc.tile_pool(name="const", bufs=1) as cpool, \
             tc.tile_pool(name="sb", bufs=2) as sb, \
             tc.tile_pool(name="wpool", bufs=3) as wp, \
             tc.tile_pool(name="persist", bufs=1) as pp, \
             tc.tile_pool(name="ps", bufs=1, space="PSUM") as ps, \
             tc.tile_pool(name="pacc", bufs=1, space="PSUM") as pacc, \
             tc.tile_pool(name="dram", bufs=1, space="DRAM") as dr:

            cst = cpool.tile([128, C_COLS], F32)
            nc.sync.dma_start(cst[:], consts[:])
            lt = cst[:, C_LT:C_LT + 128]
            ones = cst[:, C_ONES:C_ONES + 128]
            iden = cst[:, 256:384]
            iota64 = cst[:, C_IOTA64:C_IOTA64 + 64]
            biasr = cst[:, C_BIAS:C_BIAS + 64]
            iotap = cst[:, C_IOTAP:C_IOTAP + 1]
            negoff = cst[:, C_NEGOFF:C_NEGOFF + 1]
            negbig = cst[:, C_NEGBIG:C_NEGBIG + 1]
            dumpc = cst[:, C_DUMP:C_DUMP + 1]

            # DRAM intermediates
            wt_all = dr.tile([(ELOC * CAP + 128), 2], F32)  # [w, tokid] rows
            ypart = dr.tile([T + 128, H], F32)
            ag_in = dr.tile([TSH, 16], F32)
            ag_out = dr.tile([T, 16], F32, addr_space="Shared")
            rs_out = dr.tile([TSH, H], F32)

            # ---- zero init of wt table and ypart ----
            zline = pp.tile([128, H], F32)
            nc.vector.memset(zline[:], 0.0)
            for i in range(T // 128 + 1):
                nc.sync.dma_start(ypart[128 * i:128 * (i + 1), :], zline[:])
            nwt = ELOC * CAP + 128
            zwt = pp.tile([128, 2], F32, tag="zwt")
            nc.vector.memset(zwt[:, 0:1], 0.0)
            nc.vector.memset(zwt[:, 1:2], float(T))
            for i in range(nwt // 128):
                nc.sync.dma_start(
                    wt_all[128 * i:128 * (i + 1), :], zwt[:])

            # ---- transpose of this core's token slice: xT[k] = [h128, t128]
            xsh_sb = pp.tile([128, H], F32)
            nc.sync.dma_start(xsh_sb[:], xsh[:])
            xT = []
            for k in range(8):
                tp = ps.tile([128, 128], F32, space="PSUM", tag="ptrans")
                nc.tensor.transpose(out=tp[:], in_=xsh_sb[:, 128 * k:128 * (k + 1)],
                                    identity=iden)
                xt_k = pp.tile([128, 128], F32, tag=f"xT{k}")
                nc.scalar.copy(xt_k[:], tp[:])
                xT.append(xt_k)

            # ---- gate matmul: logits [128tok, 64] ----
            logits_ps = ps.tile([128, 64], F32, space="PSUM", tag="pcum")
            for k in range(8):
                gk = sb.tile([128, E], F32, tag="gwt")
                nc.sync.dma_start(gk[:], gwt[128 * k:128 * (k + 1), :])
                nc.tensor.matmul(out=logits_ps[:], lhsT=xT[k][:], rhs=gk[:],
                                 start=(k == 0), stop=(k == 7))

            # ---- routing ----
            scores = pp.tile([128, 64], F32)
            nc.scalar.activation(scores[:], logits_ps[:], AF.Sigmoid)
            sc = pp.tile([128, 64], F32)
            nc.vector.tensor_add(sc[:], scores[:], biasr)
            scg = sc[:].rearrange("p (g e) -> p g e", g=G)
            max1 = sb.tile([128, 8], F32, tag="r8")
            nc.vector.tensor_reduce(max1[:], scg, axis=mybir.AxisListType.X,
                                    op=OP.max)
            scz = sb.tile([128, 64], F32, tag="r64")
            nc.vector.match_replace(out=scz[:], in_to_replace=max1[:],
                                    in_values=sc[:], imm_value=NEG)
            max2 = sb.tile([128, 8], F32, tag="r8b")
            nc.vector.tensor_reduce(max2[:],
                                    scz[:].rearrange("p (g e) -> p g e", g=G),
                                    axis=mybir.AxisListType.X, op=OP.max)
            gsc = sb.tile([128, 8], F32, tag="r8c")
            nc.vector.tensor_add(gsc[:], max1[:], max2[:])
            gv8 = sb.tile([128, 8], F32, tag="r8d")
            nc.vector.max(out=gv8[:], in_=gsc[:])
            gmask = sb.tile([128, 8], mybir.dt.uint8, tag="r8e")
            nc.vector.tensor_tensor(out=gmask[:], in0=gsc[:],
                                    in1=gv8[:, 3:4].to_broadcast([128, 8]),
                                    op=OP.is_ge)
            emask = sb.tile([128, 64], mybir.dt.uint8, tag="emask")
            nc.vector.tensor_copy(
                emask[:].rearrange("p (g e) -> p g e", g=G),
                gmask[:, :, None].to_broadcast([128, 8, 8]))
            masked = pp.tile([128, 64], F32)
            nc.vector.tensor_copy(masked[:],
                                  negbig.to_broadcast([128, 64]))
            nc.vector.copy_predicated(masked[:], emask[:], sc[:])
            mv = sb.tile([128, 8], F32, tag="mv")
            mi = sb.tile([128, 8], U32, tag="mi")
            nc.vector.max_with_indices(mv[:], mi[:], masked[:])
            ids6 = pp.tile([128, 6], F32)
            nc.vector.tensor_copy(ids6[:], mi[:, 0:6])
            wraw = pp.tile([128, 6], F32)
            scr = sb.tile([128, 64], F32, tag="scr")
            for k in range(6):
                eqk = sb.tile([128, 64], F32, tag="eqk")
                nc.vector.tensor_tensor(
                    out=eqk[:], in0=iota64,
                    in1=ids6[:, k:k + 1].to_broadcast([128, 64]),
                    op=OP.is_equal)
                nc.vector.tensor_mul(scr[:], eqk[:], scores[:])
                nc.vector.tensor_reduce(wraw[:, k:k + 1], scr[:],
                                        axis=mybir.AxisListType.X,
                                        op=OP.add)
            wsum = sb.tile([128, 1], F32, tag="wsum")
            nc.vector.tensor_reduce(wsum[:], wraw[:],
                                    axis=mybir.AxisListType.X, op=OP.add)
            rcp = sb.tile([128, 1], F32, tag="rcp")
            nc.vector.reciprocal(rcp[:], wsum[:])
            wn = pp.tile([128, 6], F32)
            nc.vector.tensor_tensor(out=wn[:], in0=wraw[:],
                                    in1=rcp[:].to_broadcast([128, 6]),
                                    op=OP.mult)
            nc.vector.tensor_scalar_mul(wn[:], wn[:], SCALE)

            route16 = pp.tile([128, 16], F32)
            nc.vector.memset(route16[:], 0.0)
            nc.vector.tensor_copy(route16[:, 0:6], ids6[:])
            nc.vector.tensor_copy(route16[:, 6:12], wn[:])

            # ---- allgather routing ----
            if phase >= 2:
                nc.sync.dma_start(ag_in[:], route16[:])
                nc.gpsimd.collective_compute(
                    "AllGather", OP.bypass,
                    replica_groups=[list(range(N_CORES))],
                    ins=[ag_in.opt()], outs=[ag_out.opt()])

            # ---- dispatch: build wt_all via tiny scatters ----
            hits = []
            for i in range(8 if phase >= 3 else 0):
                rt = sb.tile([128, 16], F32, tag="rt")
                nc.sync.dma_start(rt[:], ag_out[128 * i:128 * (i + 1), :])
                idsT = rt[:, 0:6]
                wT = rt[:, 6:12]
                eqs = []
                hit = pp.tile([128, 64], F32, tag=f"hit{i}")
                for k in range(6):
                    eqk = sb.tile([128, 64], F32, tag="deq", bufs=8)
                    nc.vector.tensor_tensor(
                        out=eqk[:], in0=iota64,
                        in1=idsT[:, k:k + 1].to_broadcast([128, 64]),
                        op=OP.is_equal)
                    eqs.append(eqk)
                    if k == 0:
                        nc.vector.tensor_copy(hit[:], eqk[:])
                    else:
                        nc.vector.tensor_add(hit[:], hit[:], eqk[:])
                hits.append(hit)
                # cumulative count over tokens: LT@hit_i + ones@(hits j<i)
                cps = ps.tile([128, 64], F32, space="PSUM", tag="pcum")
                nc.tensor.matmul(out=cps[:], lhsT=lt, rhs=hit[:],
                                 start=True, stop=(i == 0))
                for j in range(i):
                    nc.tensor.matmul(out=cps[:], lhsT=ones, rhs=hits[j][:],
                                     start=False, stop=(j == i - 1))
                cum = sb.tile([128, 64], F32, tag="cum")
                nc.scalar.copy(cum[:], cps[:])
                # slots per k (inclusive rank), then 0-based
                slot6 = sb.tile([128, 6], F32, tag="slot6")
                scr2 = sb.tile([128, 64], F32, tag="scr2")
                for k in range(6):
                    nc.vector.tensor_mul(scr2[:], eqs[k][:], cum[:])
                    nc.vector.tensor_reduce(slot6[:, k:k + 1], scr2[:],
                                            axis=mybir.AxisListType.X,
                                            op=OP.max)
                nc.vector.tensor_scalar_add(slot6[:], slot6[:], -1.0)
                rel = sb.tile([128, 6], F32, tag="rel")
                nc.vector.tensor_tensor(out=rel[:], in0=idsT,
                                        in1=negoff.to_broadcast([128, 6]),
                                        op=OP.add)
                okA = sb.tile([128, 6], F32, tag="okA")
                nc.vector.tensor_scalar(okA[:], rel[:], 0.0, None,
                                        op0=OP.is_ge)
                okB = sb.tile([128, 6], F32, tag="okB")
                nc.vector.tensor_scalar(okB[:], rel[:], float(ELOC), None,
                                        op0=OP.is_lt)
                nc.vector.tensor_mul(okA[:], okA[:], okB[:])
                nc.vector.tensor_scalar(okB[:], slot6[:], float(CAP), None,
                                        op0=OP.is_lt)
                oki = sb.tile([128, 6], I32, tag="oki")
                nc.vector.tensor_tensor(out=oki[:], in0=okA[:], in1=okB[:],
                                        op=OP.mult)
                dest = sb.tile([128, 6], F32, tag="dest")
                nc.vector.tensor_scalar_mul(dest[:], rel[:], float(CAP))
                nc.vector.tensor_add(dest[:], dest[:], slot6[:])
                # non-ok -> DUMP
                desel = sb.tile([128, 6], F32, tag="desel")
                nc.vector.tensor_copy(desel[:], dumpc.to_broadcast([128, 6]))
                nc.vector.copy_predicated(desel[:], oki[:], dest[:])
                desti = sb.tile([128, 6], I32, tag="desti")
                nc.vector.tensor_copy(desti[:], desel[:])
                # wt rows [w, tokid]
                wt12 = sb.tile([128, 12], F32, tag="wt12")
                nc.vector.tensor_copy(
                    wt12[:].rearrange("p (a b) -> p a b", b=2)[:, :, 0:1],
                    wT[:, :, None])
                tokf = sb.tile([128, 1], F32, tag="tokf")
                nc.vector.tensor_scalar_add(tokf[:], iotap, float(128 * i))
                nc.vector.tensor_copy(
                    wt12[:].rearrange("p (a b) -> p a b", b=2)[:, :, 1:2],
                    tokf[:, :, None].to_broadcast([128, 6, 1]))
                for k in range(6):
                    nc.gpsimd.indirect_dma_start(
                        out=wt_all[:],
                        out_offset=bass.IndirectOffsetOnAxis(
                            ap=desti[:, k:k + 1], axis=0),
                        in_=wt12[:, 2 * k:2 * k + 2], in_offset=None)

            # ---- shared expert on own slice ----
            do_shared = phase >= 4
            gus = [pacc.tile([128, 512], F32, space="PSUM", tag=f"pg{n}",
                             name=f"pgus{n}") for n in range(4)]
            for k in range(8 if do_shared else 0):
                swk = wp.tile([128, 2 * FSH], F32, tag="sw13")
                nc.sync.dma_start(swk[:], sw13[128 * k:128 * (k + 1), :])
                for n in range(4):
                    nc.tensor.matmul(out=gus[n][:], lhsT=xT[k][:],
                                     rhs=swk[:, 512 * n:512 * (n + 1)],
                                     start=(k == 0), stop=(k == 7))
            acts = []
            for n in range(2 if do_shared else 0):
                sil = sb.tile([128, 512], F32, tag="ssil")
                nc.scalar.activation(sil[:], gus[n][:], AF.Sigmoid)
                nc.vector.tensor_mul(sil[:], sil[:], gus[n][:])
                ac = pp.tile([128, 512], F32, tag=f"sact{n}")
                nc.vector.tensor_mul(ac[:], sil[:], gus[n + 2][:])
                acts.append(ac)
            actT = []
            for k in range(8 if do_shared else 0):
                tp = ps.tile([128, 128], F32, space="PSUM", tag="ptrans")
                src = acts[k // 4]
                nc.tensor.transpose(
                    out=tp[:], in_=src[:, 128 * (k % 4):128 * (k % 4 + 1)],
                    identity=iden)
                at = pp.tile([128, 128], F32, tag=f"sactT{k}")
                nc.scalar.copy(at[:], tp[:])
                actT.append(at)
            ysps = [pacc.tile([128, 512], F32, space="PSUM", tag=f"pe{n}",
                              name=f"pys{n}") for n in range(2)]
            for k in range(8 if do_shared else 0):
                s2k = wp.tile([128, H], F32, tag="sw2")
                nc.sync.dma_start(s2k[:], sw2[128 * k:128 * (k + 1), :])
                for n in range(2):
                    nc.tensor.matmul(out=ysps[n][:], lhsT=actT[k][:],
                                     rhs=s2k[:, 512 * n:512 * (n + 1)],
                                     start=(k == 0), stop=(k == 7))
            shared = pp.tile([128, H], F32)
            if do_shared:
                for n in range(2):
                    nc.scalar.copy(shared[:, 512 * n:512 * (n + 1)],
                                   ysps[n][:])
            else:
                nc.vector.memset(shared[:], 0.0)

            # ---- experts ----
            for e in range(ELOC if phase >= 5 else 0):
                wts, toki, wv = [], [], []
                for j in range(CTILES):
                    wt = sb.tile([128, 2], F32, tag="wt", bufs=5)
                    r0 = e * CAP + 128 * j
                    nc.sync.dma_start(wt[:], wt_all[r0:r0 + 128, :])
                    ti = sb.tile([128, 1], I32, tag="toki", bufs=5)
                    nc.vector.tensor_copy(ti[:], wt[:, 1:2])
                    tg = sb.tile([128, 1], F32, tag="tokg", bufs=5)
                    nc.vector.tensor_scalar_min(tg[:], wt[:, 1:2],
                                                float(T - 1))
                    tgi = sb.tile([128, 1], I32, tag="tokgi", bufs=5)
                    nc.vector.tensor_copy(tgi[:], tg[:])
                    wts.append(wt)
                    toki.append((ti, tgi))
                    wv.append(wt[:, 0:1])
                # gather + transpose x rows
                xgT = [[None] * CTILES for _ in range(8)]
                for j in range(CTILES):
                    xg = sb.tile([128, H], F32, tag="xg")
                    nc.gpsimd.indirect_dma_start(
                        out=xg[:], out_offset=None, in_=x[:],
                        in_offset=bass.IndirectOffsetOnAxis(
                            ap=toki[j][1][:, 0:1], axis=0))
                    for k in range(8):
                        tp = ps.tile([128, 128], F32, space="PSUM",
                                     tag="ptrans")
                        nc.tensor.transpose(
                            out=tp[:], in_=xg[:, 128 * k:128 * (k + 1)],
                            identity=iden)
                        xt_ = sb.tile([128, 128], F32, tag=f"xgT{k}_{j}")
                        nc.scalar.copy(xt_[:], tp[:])
                        xgT[k][j] = xt_
                # mm1: gu[j] = xg[j] @ w13_e   -> [128, 1024]
                gu = [[pacc.tile([128, 512], F32, space="PSUM", tag=f"pg{2 * j + n}",
                                 name=f"pgu{j}{n}") for n in range(2)]
                      for j in range(CTILES)]
                for k in range(8):
                    wk = wp.tile([128, 2 * F], F32, tag="w13")
                    nc.sync.dma_start(wk[:], w13[e, 128 * k:128 * (k + 1), :])
                    for j in range(CTILES):
                        for n in range(2):
                            nc.tensor.matmul(
                                out=gu[j][n][:], lhsT=xgT[k][j][:],
                                rhs=wk[:, 512 * n:512 * (n + 1)],
                                start=(k == 0), stop=(k == 7))
                # silu * mul -> act[j] [128, 512]
                act = []
                for j in range(CTILES):
                    sil = sb.tile([128, 512], F32, tag="esil")
                    nc.scalar.activation(sil[:], gu[j][0][:], AF.Sigmoid)
                    nc.vector.tensor_mul(sil[:], sil[:], gu[j][0][:])
                    ac = sb.tile([128, 512], F32, tag=f"eact{j}")
                    nc.vector.tensor_mul(ac[:], sil[:], gu[j][1][:])
                    act.append(ac)
                # transpose act -> actTe[k][j]
                actTe = [[None] * CTILES for _ in range(4)]
                for j in range(CTILES):
                    for k in range(4):
                        tp = ps.tile([128, 128], F32, space="PSUM",
                                     tag="ptrans")
                        nc.tensor.transpose(
                            out=tp[:], in_=act[j][:, 128 * k:128 * (k + 1)],
                            identity=iden)
                        at = sb.tile([128, 128], F32, tag=f"actT{k}_{j}")
                        nc.scalar.copy(at[:], tp[:])
                        actTe[k][j] = at
                # mm2: eo[j] = act[j] @ w2_e -> [128, 1024]
                w2sb = []
                for k in range(4):
                    w2k = wp.tile([128, H], F32, tag=f"w2_{k}", bufs=2,
                                  name=f"w2k{k}")
                    nc.sync.dma_start(w2k[:], w2[e, 128 * k:128 * (k + 1), :])
                    w2sb.append(w2k)
                for j in range(CTILES):
                    eo = [pacc.tile([128, 512], F32, space="PSUM",
                                    tag=f"pe{n}", name=f"peo{n}")
                          for n in range(2)]
                    for k in range(4):
                        for n in range(2):
                            nc.tensor.matmul(
                                out=eo[n][:], lhsT=actTe[k][j][:],
                                rhs=w2sb[k][:, 512 * n:512 * (n + 1)],
                                start=(k == 0), stop=(k == 3))
                    eos = sb.tile([128, H], F32, tag="eos")
                    for n in range(2):
                        nc.scalar.activation(
                            eos[:, 512 * n:512 * (n + 1)], eo[n][:],
                            AF.Copy, scale=wv[j])
                    nc.gpsimd.indirect_dma_start(
                        out=ypart[:],
                        out_offset=bass.IndirectOffsetOnAxis(
                            ap=toki[j][0][:, 0:1], axis=0),
                        in_=eos[:], in_offset=None,
                        compute_op=OP.add)

            # ---- reduce-scatter + shared add ----
            if phase >= 6:
                nc.gpsimd.collective_compute(
                    "ReduceScatter", OP.add,
                    replica_groups=[list(range(N_CORES))],
                    ins=[ypart[0:T, :]], outs=[rs_out.opt()])
                rssb = pp.tile([128, H], F32, tag="rssb")
                nc.sync.dma_start(rssb[:], rs_out[:])
                yfin = pp.tile([128, H], F32, tag="yfin")
                nc.vector.tensor_add(yfin[:], rssb[:], shared[:])
                nc.sync.dma_start(yout[:], yfin[:])
            elif phase == 5:
                pr = pp.tile([128, H], F32, tag="prr")
                nc.sync.dma_start(pr[:], ypart[0:128, :])
                nc.sync.dma_start(yout[:], pr[:])
            elif phase == 4:
                nc.sync.dma_start(yout[:], shared[:])
            elif phase == 3:
                wtr = pp.tile([128, 2], F32, tag="wtr")
                nc.sync.dma_start(wtr[:], wt_all[0:128, :])
                yz = pp.tile([128, H], F32, tag="yz")
                nc.vector.memset(yz[:], 0.0)
                nc.vector.tensor_copy(yz[:, 0:2], wtr[:])
                nc.sync.dma_start(yout[:], yz[:])
            elif phase == 2:
                agr = pp.tile([128, 16], F32, tag="agr")
                nc.sync.dma_start(agr[:], ag_out[0:128, :])
                yz = pp.tile([128, H], F32, tag="yz")
                nc.vector.memset(yz[:], 0.0)
                nc.vector.tensor_copy(yz[:, 0:16], agr[:])
                nc.sync.dma_start(yout[:], yz[:])
            else:
                yz = pp.tile([128, H], F32, tag="yz")
                nc.vector.memset(yz[:], 0.0)
                nc.vector.tensor_copy(yz[:, 0:16], route16[:])
                nc.sync.dma_start(yout[:], yz[:])

    nc.compile()
    return nc


def make_consts(core: int, bias: np.ndarray) -> np.ndarray:
    c = np.zeros((128, C_COLS), np.float32)
    a = np.arange(128)
    c[:, 0:128] = (a[:, None] <= a[None, :]).astype(np.float32)  # LT
    c[:, 128:256] = 1.0                                          # ONES
    c[:, 256:384] = np.eye(128, dtype=np.float32)                # IDEN
    c[:, C_IOTA64:C_IOTA64 + 64] = np.arange(64, dtype=np.float32)[None, :]
    c[:, C_BIAS:C_BIAS + 64] = bias.astype(np.float32)[None, :]
    c[:, C_IOTAP] = a.astype(np.float32)
    c[:, C_NEGOFF] = -float(ELOC * core)
    c[:, C_NEGBIG] = NEG
    c[:, C_DUMP] = float(DUMP)
    return c


def kernel(**inputs) -> np.ndarray:
    x = np.ascontiguousarray(np.asarray(inputs["hidden_states"], np.float32))
    gate_w = np.asarray(inputs["gate_w"], np.float32)
    bias = np.asarray(inputs["bias"], np.float32)
    w13 = np.asarray(inputs["w13"], np.float32)
    w2 = np.asarray(inputs["w2"], np.float32)
    sw13 = np.ascontiguousarray(np.asarray(inputs["sw13"], np.float32))
    sw2 = np.ascontiguousarray(np.asarray(inputs["sw2"], np.float32))
    gwt = np.ascontiguousarray(gate_w.T)

    if "nc" not in _CACHE:
        _CACHE["nc"] = build_nc()
    nc = _CACHE["nc"]

    in_maps = []
    for c in range(N_CORES):
        in_maps.append({
            "x": x,
            "xsh": x[TSH * c:TSH * (c + 1), :],
            "gwt": gwt,
            "w13": np.ascontiguousarray(w13[ELOC * c:ELOC * (c + 1)]),
            "w2": np.ascontiguousarray(w2[ELOC * c:ELOC * (c + 1)]),
            "sw13": sw13,
            "sw2": sw2,
            "consts": make_consts(c, bias),
        })
    res = bass_utils.run_bass_kernel_spmd(
        nc, in_maps, core_ids=list(range(N_CORES)))
    out = np.concatenate([res.results[c]["y"] for c in range(N_CORES)],
                         axis=0)
    return out.astype(np.float32)


if __name__ == "__main__":
    xs = {k: np.load(f"/tmp/inp_{k}.npy") for k in
          ["hidden_states", "gate_w", "bias", "w13", "w2", "sw13", "sw2"]}
    got = kernel(**xs)
    exp = np.load("/tmp/ref_out.npy")
    err = np.linalg.norm(got - exp) / np.linalg.norm(exp)
    print("Relative error:", err)
    print("max abs err:", np.abs(got - exp).max())
